# revision 73
# baseline (speedup 1.0000x reference)
"""Causal GQA attention (B=2, T=2048, D=2048, QH=16, KVH=4, HD=128) on 8 TRN2 cores.

Sharding: DP-2 over batch x TP-4 over KV-head groups.
  core c -> batch c//4, kv head c%4, q heads 4*(c%4)..4*(c%4)+3.
Each core computes a partial (T, D) output (its heads' contribution through wo);
the host sums the 4 partials per batch and stacks the two batches.

Device dataflow (everything transposed; no on-device activation transposes):
  - host feeds xT = x[b].T (D, T); projections computed directly in [hd, t]
    layout; RoPE via swap-permutation matmul + DVE mul/add
  - scores per PAIR of 128-key blocks into one 2-bank PSUM tile [128,2,512];
    ONE ACT exp instruction covers the pair
  - causal: fully-masked column ranges never computed; diagonal 128x128
    triangle masked by a DVE multiply
  - softmax denominator: DVE accumulates exp sums per head in SBUF (fp16),
    then ONE ones@acc matmul per (head, tile) broadcasts it -- removes the
    per-pair ones@expS matmuls from the PE (~29k columns saved)
  - WEAVE: the projection matmuls of tile tt+1 (and for the last tile, the
    output projection of tile 2) are emitted interleaved between attention
    pairs of tile tt, so the PE never stalls on the exp latency and the
    ACT engine's work hides entirely under PE work
  - PSUM plan: pp 2x[128,2,512] (score pairs + den rides the ring),
    po 2x[128,512] (o accumulators, deferred-norm depth 2),
    pa 1x[128,2,512] (weave slot: proj/rope/vt psums, woven wo pairs)
  - kT/v stored in per-tile tiles so tile tt+1's writes don't false-WAR
    against tile tt's attention reads
  - out[t, d] = sum_h (OT_h)^T @ wo_h in paired psum tiles, evac fp16,
    one DMA per pair
  - ~16 dummy matmuls at t=0 lift the HAM clock gate early
All matmuls fp16 (1 cycle/row on the PE; fp32 PSUM accumulation).
"""
import numpy as np
from contextlib import ExitStack

import concourse.bacc as bacc
import concourse.tile as tile
import concourse.mybir as mybir
from concourse.bass_utils import run_bass_kernel_spmd

B, T, D = 2, 2048, 2048
QH, KVH = 16, 4
HD = D // QH            # 128
P = 128
NT = T // 512           # 4 t-tiles of 512
DC = D // P             # 16 contraction chunks
KB = T // P             # 16 key blocks
F32 = mybir.dt.float32
CDT = mybir.dt.float16          # compute dtype on the PE (1 cycle/row)
NPDT = np.float16
AF = mybir.ActivationFunctionType
SCALE = float(1.0 / np.sqrt(HD))

_cached = {}


def _build():
    nc = bacc.Bacc("TRN2", target_bir_lowering=False, debug=False)
    # pre-shuffled on host so every DMA line is >=4KB contiguous
    xT = nc.dram_tensor("xT", [NT, 4, P, 4, 512], CDT, kind="ExternalInput")
    wq = nc.dram_tensor("wq", [4, P, 4, 512], CDT, kind="ExternalInput")
    wk = nc.dram_tensor("wk", [P, DC, HD], CDT, kind="ExternalInput")
    wv = nc.dram_tensor("wv", [P, DC, HD], CDT, kind="ExternalInput")
    wo = nc.dram_tensor("wo", [4 * HD, D], CDT, kind="ExternalInput")
    cosT = nc.dram_tensor("cosT", [HD, T], CDT, kind="ExternalInput")
    ssinT = nc.dram_tensor("ssinT", [HD, T], CDT, kind="ExternalInput")
    rmat = nc.dram_tensor("rmat", [P, P], CDT, kind="ExternalInput")
    tri = nc.dram_tensor("tri", [P, P], CDT, kind="ExternalInput")
    ident = nc.dram_tensor("ident", [P, P], CDT, kind="ExternalInput")
    out = nc.dram_tensor("out", [T, D], CDT, kind="ExternalOutput")

    with tile.TileContext(nc) as tc, ExitStack() as ctx:
        const = ctx.enter_context(tc.tile_pool(name="const", bufs=1))
        kvres = ctx.enter_context(tc.tile_pool(name="kvres", bufs=1))
        xc_pool = ctx.enter_context(tc.tile_pool(name="xc", bufs=8))
        qr_pool = ctx.enter_context(tc.tile_pool(name="qr", bufs=8))
        tmp_pool = ctx.enter_context(tc.tile_pool(name="tmp", bufs=8))
        e_pool = ctx.enter_context(tc.tile_pool(name="ep", bufs=8))
        da_pool = ctx.enter_context(tc.tile_pool(name="da", bufs=2))
        ot_pool = ctx.enter_context(tc.tile_pool(name="ot", bufs=2))
        oev_pool = ctx.enter_context(tc.tile_pool(name="oev", bufs=6))
        bc_pool = ctx.enter_context(tc.tile_pool(name="bc", bufs=3))

        # PSUM: 8 banks.  pp = 2x[128,2,512] score pairs (den matmul rides the
        # ring); po = 2x[128,512] o accumulators; pa = 1x[128,2,512] weave slot
        pp = ctx.enter_context(tc.tile_pool(name="pp", bufs=2, space="PSUM"))
        po = ctx.enter_context(tc.tile_pool(name="po", bufs=2, space="PSUM"))
        pa = ctx.enter_context(tc.tile_pool(name="pa", bufs=1, space="PSUM"))

        # ---- resident constants ----
        wq_sb = const.tile([P, DC, 4 * HD], CDT, tag="wq")
        wk_sb = const.tile([P, DC, HD], CDT, tag="wk")
        wv_sb = const.tile([P, DC, HD], CDT, tag="wv")
        wo_sb = const.tile([P, 4, D], CDT, tag="wo")
        cos_sb = const.tile([P, T], CDT, tag="cos")
        sin_sb = const.tile([P, T], CDT, tag="sin")
        rm_sb = const.tile([P, P], CDT, tag="rm")
        tri_sb = const.tile([P, P], CDT, tag="tri")
        id_sb = const.tile([P, P], CDT, tag="id")
        ones_mat = const.tile([P, P], CDT, tag="ones")
        dum_sb = const.tile([P, 256], CDT, tag="dum")

        # per-tile K^T / V tiles (separate tags -> no cross-tile false deps)
        kT_t = [kvres.tile([P, 512], CDT, tag=f"kT{t_}", name=f"kT{t_}")
                for t_ in range(NT)]
        v_t = [kvres.tile([P, 4, HD], CDT, tag=f"V{t_}", name=f"V{t_}")
               for t_ in range(NT)]

        wo_v = wo.rearrange("(c p) n -> c p n", p=P)

        # ---- HAM warmup: PE busy from t=0 so the clock gate lifts early ----
        nc.vector.memset(dum_sb[:], 0.0)
        dum_ps = pa.tile([P, 2, 512], F32, tag="pa", name="dum_ps")
        for i in range(16):
            nc.tensor.matmul(dum_ps[:, 0, 0:256], dum_sb[:, 0:128],
                             dum_sb[:, 0:256], start=True, stop=True,
                             skip_group_check=True)

        # ---- startup DMAs for tile 0, paced per-chunk ----
        xg0 = []
        for g in range(4):
            xg = xc_pool.tile([P, 4, 512], CDT, tag="xc", name=f"xc0_{g}")
            xg0.append(xg)
        # the startup-critical 4MB (x0 + wq) interleaved per half-group
        # (2KB lines) across both queues, in consumption order
        nc.sync.dma_start(out=rm_sb[:], in_=rmat[:])
        nc.gpsimd.dma_start(out=id_sb[:], in_=ident[:])
        nc.sync.dma_start(out=xg0[0][:, 0, :], in_=xT[0, 0, :, 0, :])
        nc.gpsimd.dma_start(out=wq_sb[:, 0, :], in_=wq[0, :, 0, :])
        nc.sync.dma_start(out=xg0[0][:, 1, :], in_=xT[0, 0, :, 1, :])
        nc.gpsimd.dma_start(out=wq_sb[:, 1, :], in_=wq[0, :, 1, :])
        nc.vector.memset(ones_mat[:], 1.0)
        for p in range(1, 8):
            g, j = p // 2, (p % 2) * 2
            xq = nc.sync if p % 2 == 0 else nc.gpsimd
            wqq = nc.gpsimd if p % 2 == 0 else nc.sync
            xq.dma_start(out=xg0[g][:, j:j + 2, :], in_=xT[0, g, :, j:j + 2, :])
            wqq.dma_start(out=wq_sb[:, g * 4 + j:g * 4 + j + 2, :],
                          in_=wq[g, :, j:j + 2, :])
        nc.sync.dma_start(out=wk_sb[:], in_=wk[:])
        nc.gpsimd.dma_start(out=wv_sb[:], in_=wv[:])
        nc.sync.dma_start(out=cos_sb[:], in_=cosT[:])
        nc.gpsimd.dma_start(out=sin_sb[:], in_=ssinT[:])

        x_tiles = {0: xg0}

        def load_x_tile(tt, q=None):
            q = q or nc.gpsimd
            grps = []
            for g in range(4):
                xg = xc_pool.tile([P, 4, 512], CDT, tag="xc", name=f"xc{tt}_{g}")
                q.dma_start(out=xg[:], in_=xT[tt, g])
                grps.append(xg)
            x_tiles[tt] = grps

        qr_tiles = {}

        def make_qr(tt):
            qr_tiles[tt] = [qr_pool.tile([P, 512], CDT, tag="qr",
                                         name=f"qr{tt}_{i}") for i in range(4)]

        def rope_muls(dst_ap, sb, rot_ap, tt, nm, eng=None):
            """dst = sb*cos + rot*ssin, fp16.  rot_ap reads PSUM so its mul
            stays on DVE; the SBUF-only mul/add can offload to gpsimd."""
            eng = nc.vector
            c_sl = cos_sb[:, tt * 512:(tt + 1) * 512]
            s_sl = sin_sb[:, tt * 512:(tt + 1) * 512]
            t1 = tmp_pool.tile([P, 512], CDT, tag="t1", name=f"t1_{nm}")
            t2 = tmp_pool.tile([P, 512], CDT, tag="t2", name=f"t2_{nm}")
            with nc.allow_low_precision(reason="fp16 rope"):
                eng.tensor_mul(t2[:], rot_ap, s_sl)
                eng.tensor_mul(t1[:], sb[:], c_sl)
                eng.tensor_add(dst_ap, t1[:], t2[:])

        # ---------------- A0: projection of tile 0 (inline, DMA-paced) ------
        q01 = pp.tile([P, 2, 512], F32, tag="pair", name="a0_q01")
        q23 = pp.tile([P, 2, 512], F32, tag="pair", name="a0_q23")
        for dc in range(DC):
            xc = xg0[dc // 4][:, dc % 4, :]
            st, sp = (dc == 0), (dc == DC - 1)
            for idx in range(4):
                tgt = q01 if idx < 2 else q23
                nc.tensor.matmul(tgt[:, idx % 2, :],
                                 wq_sb[:, dc, idx * HD:(idx + 1) * HD],
                                 xc, start=st, stop=sp)
        make_qr(0)
        # evac q0..q3, rope via po ring; kv chunks interleave to cover the
        # DVE evac/mul latencies with PE work
        sbq = []
        for i in range(2):
            sb = tmp_pool.tile([P, 512], CDT, tag="ev", name=f"a0_ev{i}")
            with nc.allow_low_precision(reason="evac"):
                nc.vector.tensor_copy(sb[:], q01[:, i, :])
            sbq.append(sb)
        kv_ps = pa.tile([P, 2, 512], F32, tag="pa", name="a0_kv")
        for dc in range(8):
            xc = xg0[dc // 4][:, dc % 4, :]
            nc.tensor.matmul(kv_ps[:, 0, :], wk_sb[:, dc, :], xc,
                             start=(dc == 0), stop=False)
            nc.tensor.matmul(kv_ps[:, 1, :], wv_sb[:, dc, :], xc,
                             start=(dc == 0), stop=False)
        for i in range(2):
            rot = po.tile([P, 512], F32, tag="o", name=f"a0_rot{i}")
            nc.tensor.matmul(rot[:], rm_sb[:], sbq[i][:], start=True, stop=True)
            rope_muls(qr_tiles[0][i][:], sbq[i], rot[:], 0, f"a0q{i}")
        for i in range(2, 4):
            sb = tmp_pool.tile([P, 512], CDT, tag="ev", name=f"a0_ev{i}")
            with nc.allow_low_precision(reason="evac"):
                nc.vector.tensor_copy(sb[:], q23[:, i - 2, :])
            sbq.append(sb)
        for dc in range(8, DC):
            xc = xg0[dc // 4][:, dc % 4, :]
            st, sp = False, (dc == DC - 1)
            nc.tensor.matmul(kv_ps[:, 0, :], wk_sb[:, dc, :], xc,
                             start=st, stop=sp)
            nc.tensor.matmul(kv_ps[:, 1, :], wv_sb[:, dc, :], xc,
                             start=st, stop=sp)
        for i in range(2, 4):
            rot = po.tile([P, 512], F32, tag="o", name=f"a0_rot{i}")
            nc.tensor.matmul(rot[:], rm_sb[:], sbq[i][:], start=True, stop=True)
            rope_muls(qr_tiles[0][i][:], sbq[i], rot[:], 0, f"a0q{i}")
        sbk = tmp_pool.tile([P, 512], CDT, tag="ev", name="a0_evk")
        sbv = tmp_pool.tile([P, 512], CDT, tag="ev", name="a0_evv")
        with nc.allow_low_precision(reason="evac"):
            nc.vector.tensor_copy(sbk[:], kv_ps[:, 0, :])
        nc.scalar.copy(sbv[:], kv_ps[:, 1, :])
        rotk = po.tile([P, 512], F32, tag="o", name="a0_rotk")
        nc.tensor.matmul(rotk[:], rm_sb[:], sbk[:], start=True, stop=True)
        rope_muls(kT_t[0][:], sbk, rotk[:], 0, "a0k")
        # prefetch x tile 1 + tri + wo now
        load_x_tile(1, q=nc.sync)
        nc.sync.dma_start(out=tri_sb[:], in_=tri[:])
        for c in range(4):
            nc.gpsimd.dma_start(out=wo_sb[:, c, :], in_=wo_v[c])

        def a0_tail_items():
            st0 = {}

            def vtrans0():
                vt = pa.tile([P, 512], CDT, tag="pa", name="a0_vt")
                for i in range(4):
                    nc.tensor.transpose(vt[:, i * P:(i + 1) * P],
                                        sbv[:, i * P:(i + 1) * P], id_sb[:])
                st0['vt'] = vt

            def vstore0():
                for i in range(4):
                    with nc.allow_low_precision(reason="fp16 store"):
                        nc.vector.tensor_copy(v_t[0][:, i, :],
                                              st0['vt'][:, i * P:(i + 1) * P])
            return [(2048, vtrans0), (2048, vstore0)]

        # ---------------- weave machinery ----------------
        fillers = []          # list of (cost_cols, closure)

        def pop_fill(target):
            done = 0
            while fillers and done < target:
                cost, fn = fillers.pop(0)
                fn()
                done += cost
            return done

        def a_items(tt, split_q23=False):
            """Projection of tile tt as weave closures (x already resident).
            split_q23: return (main_items, q23_items) so the q23 chain can be
            woven into the NEXT tile's attention instead."""
            items = []
            st_ = {}
            make_qr(tt)
            xgs = x_tiles[tt]

            def qpass(which, grp):   # which in ('q01','q23'), grp 0..3
                def fn():
                    if grp == 0:
                        st_[which] = pa.tile([P, 2, 512], F32, tag="pa",
                                             name=f"a{tt}_{which}")
                    ps = st_[which]
                    base = 0 if which == 'q01' else 2
                    for dc in range(grp * 4, grp * 4 + 4):
                        xc = xgs[dc // 4][:, dc % 4, :]
                        st, sp = (dc == 0), (dc == DC - 1)
                        for j in range(2):
                            idx = base + j
                            nc.tensor.matmul(ps[:, j, :],
                                             wq_sb[:, dc,
                                                   idx * HD:(idx + 1) * HD],
                                             xc, start=st, stop=sp)
                return fn

            def qevac(which):
                def fn():
                    base = 0 if which == 'q01' else 2
                    for j in range(2):
                        sb = tmp_pool.tile([P, 512], CDT, tag="ev",
                                           name=f"a{tt}_ev{base + j}")
                        with nc.allow_low_precision(reason="evac"):
                            nc.vector.tensor_copy(sb[:], st_[which][:, j, :])
                        st_[f"sb{base + j}"] = sb
                return fn

            def qrot(which):
                def fn():
                    base = 0 if which == 'q01' else 2
                    rot = pa.tile([P, 2, 512], F32, tag="pa",
                                  name=f"a{tt}_rot{which}")
                    for j in range(2):
                        nc.tensor.matmul(rot[:, j, :], rm_sb[:],
                                         st_[f"sb{base + j}"][:],
                                         start=True, stop=True)
                    st_[f"rot{which}"] = rot
                return fn

            def qmul(which, j):
                def fn():
                    base = 0 if which == 'q01' else 2
                    rope_muls(qr_tiles[tt][base + j][:], st_[f"sb{base + j}"],
                              st_[f"rot{which}"][:, j, :], tt,
                              f"a{tt}q{base + j}", eng=nc.gpsimd)
                return fn

            def kvpass(grp):
                def fn():
                    if grp == 0:
                        st_['kv'] = pa.tile([P, 2, 512], F32, tag="pa",
                                            name=f"a{tt}_kv")
                    ps = st_['kv']
                    for dc in range(grp * 4, grp * 4 + 4):
                        xc = xgs[dc // 4][:, dc % 4, :]
                        st, sp = (dc == 0), (dc == DC - 1)
                        nc.tensor.matmul(ps[:, 0, :], wk_sb[:, dc, :], xc,
                                         start=st, stop=sp)
                        nc.tensor.matmul(ps[:, 1, :], wv_sb[:, dc, :], xc,
                                         start=st, stop=sp)
                return fn

            def kvevac():
                def fn():
                    sbk = tmp_pool.tile([P, 512], CDT, tag="ev",
                                        name=f"a{tt}_evk")
                    sbv = tmp_pool.tile([P, 512], CDT, tag="ev",
                                        name=f"a{tt}_evv")
                    with nc.allow_low_precision(reason="evac"):
                        nc.vector.tensor_copy(sbk[:], st_['kv'][:, 0, :])
                        nc.vector.tensor_copy(sbv[:], st_['kv'][:, 1, :])
                    st_['sbk'], st_['sbv'] = sbk, sbv
                return fn

            def krot():
                def fn():
                    rk = pa.tile([P, 2, 512], F32, tag="pa",
                                 name=f"a{tt}_rk")
                    nc.tensor.matmul(rk[:, 0, :], rm_sb[:], st_['sbk'][:],
                                     start=True, stop=True)
                    st_['rk'] = rk
                return fn

            def kmul():
                def fn():
                    rope_muls(kT_t[tt][:], st_['sbk'], st_['rk'][:, 0, :],
                              tt, f"a{tt}k", eng=nc.gpsimd)
                return fn

            def vtrans():
                def fn():
                    vt = pa.tile([P, 512], CDT, tag="pa", name=f"a{tt}_vt")
                    for i in range(4):
                        nc.tensor.transpose(vt[:, i * P:(i + 1) * P],
                                            st_['sbv'][:, i * P:(i + 1) * P],
                                            id_sb[:])
                    st_['vt'] = vt
                return fn

            def vstore():
                def fn():
                    for i in range(4):
                        with nc.allow_low_precision(reason="fp16 store"):
                            nc.vector.tensor_copy(
                                v_t[tt][:, i, :],
                                st_['vt'][:, i * P:(i + 1) * P])
                return fn

            # order: q01 -> (rope q0/q1) -> kv -> (k rope, v transpose) ->
            # q23 -> (rope q2/q3).  kT/v land mid-window; q heads 2/3 (only
            # needed halfway through the NEXT tile's attention) land last.
            # pseudo-costs put each dependent closure in a later pop batch
            # than its producer so cross-engine latency is covered by B pairs.
            for g in range(4):
                items.append((4096, qpass('q01', g)))
            items.append((4096, qevac('q01')))
            items.append((2048, qrot('q01')))
            items.append((1024, qmul('q01', 0)))
            items.append((1024, qmul('q01', 1)))
            for g in range(4):
                items.append((4096, kvpass(g)))
            items.append((4096, kvevac()))
            items.append((2048, krot()))
            items.append((1024, kmul()))
            items.append((2048, vtrans()))
            items.append((1024, vstore()))
            q23_items = []
            for g in range(4):
                q23_items.append((4096, qpass('q23', g)))
            q23_items.append((4096, qevac('q23')))
            q23_items.append((2048, qrot('q23')))
            q23_items.append((1024, qmul('q23', 0)))
            q23_items.append((1024, qmul('q23', 1)))
            if split_q23:
                return items, q23_items
            return items + q23_items

        ot_tiles = {}

        def c_items(tt, pool, eager_tail=False):
            """Output projection of tile tt as closures (f2 from `pool`)."""
            items = []
            ot_sb = ot_tiles[tt]

            def group(tc4, dp):
                def fn():
                    trow = tt * 512 + tc4 * P
                    f2 = pool.tile([P, 2, 512], F32,
                                   tag="pair" if pool is pp else "pa",
                                   name=f"f{tt}_{tc4}_{dp}")
                    for hh in range(4):
                        for half in range(2):
                            doc = dp * 2 + half
                            nc.tensor.matmul(
                                f2[:, half, :],
                                ot_sb[:, hh, tc4 * P:(tc4 + 1) * P],
                                wo_sb[:, hh, doc * 512:(doc + 1) * 512],
                                start=(hh == 0), stop=(hh == 3))
                    o_ev = oev_pool.tile([P, 2, 512], CDT, tag="oev",
                                         name=f"oe{tt}_{tc4}_{dp}")
                    if eager_tail:
                        # keep the HAM activity monitor fed so the clock
                        # gate stays lifted through the final evac/DMA chain
                        dmy = po.tile([P, 512], F32, tag="o",
                                      name=f"dmy{tc4}_{dp}")
                        nc.tensor.matmul(dmy[:, 0:128], dum_sb[:, 0:128],
                                         dum_sb[:, 0:128], start=True,
                                         stop=True, skip_group_check=True)
                    if eager_tail and tc4 == 3:
                        with nc.allow_low_precision(reason="fp16 out"):
                            nc.vector.tensor_copy(o_ev[:, 0, :], f2[:, 0, :])
                            nc.scalar.copy(o_ev[:, 1, :], f2[:, 1, :])
                        # spread the four final DMAs across three queues so
                        # their issue slots don't serialize on the tail
                        qa = nc.sync if dp == 0 else nc.scalar
                        qb = nc.gpsimd if dp == 0 else nc.sync
                        qa.dma_start(
                            out=out[trow:trow + P, dp * 1024:dp * 1024 + 512],
                            in_=o_ev[:, 0, :])
                        qb.dma_start(
                            out=out[trow:trow + P,
                                    dp * 1024 + 512:(dp + 1) * 1024],
                            in_=o_ev[:, 1, :])
                        return
                    with nc.allow_low_precision(reason="fp16 out"):
                        # woven C (pa pool): keep the ACT queue exp-only
                        if pool is pa or dp % 2 == 0:
                            nc.vector.tensor_copy(o_ev[:], f2[:])
                        else:
                            nc.scalar.copy(o_ev[:], f2[:])
                    if eager_tail:
                        qs = [nc.sync, nc.gpsimd, nc.scalar]
                        q = qs[(tc4 * 2 + dp) % 3]
                    else:
                        q = nc.sync if (tc4 + dp) % 2 == 0 else nc.gpsimd
                    q.dma_start(out=out[trow:trow + P,
                                        dp * 1024:(dp + 1) * 1024],
                                in_=o_ev[:])
                return fn

            for tc4 in range(4):
                for dp in range(2):
                    items.append((4096, group(tc4, dp)))
            return items

        # ---------------- phase B: attention for tile tt (woven) ------------
        def finish2(bc_sb, o_ps, hh, ot_sb):
            rb_sb = bc_pool.tile([P, 512], F32, tag="rb", name=f"rb_{hh}")
            nc.vector.reciprocal_approx_fast(out=rb_sb[:], in_=bc_sb[:])
            with nc.allow_low_precision(reason="norm"):
                nc.vector.tensor_mul(ot_sb[:, hh, :], o_ps[:], rb_sb[:])

        def phase_b(tt, due):
            npair = 2 * (tt + 1)
            ot_sb = ot_pool.tile([P, 4, 512], CDT, tag="ot", name=f"ot{tt}")
            ot_tiles[tt] = ot_sb
            if tt == 0:
                for _ in range(10):
                    nc.tensor.ldweights(weights=dum_sb[:, 0:128])
            total_fill = sum(c for c, _ in fillers)
            npops = 8 * npair + 12
            per_pop = max(256, total_fill // max(1, npops) + 1)

            for hh in range(4):
                o_ps = po.tile([P, 512], F32, tag="o", name=f"o{tt}_{hh}")
                acc = da_pool.tile([P, 512], CDT, tag="da", name=f"da{tt}_{hh}")
                pend = []
                for pj in range(npair):
                    # previous head's deferred den / norm land here, long
                    # after their exp/DVE producer chains have completed
                    if pj == min(2, npair - 1) and due:
                        due.pop(0)()
                    if pj == 3 and due:
                        due.pop(0)()
                    kb0, kb1 = 2 * pj, 2 * pj + 1
                    d0, d1 = kb0 - 4 * tt, kb1 - 4 * tt
                    lo0 = d0 * P if d0 > 0 else 0
                    lo1 = d1 * P if d1 > 0 else 0
                    ps2 = pp.tile([P, 2, 512], F32, tag="pair",
                                  name=f"s{tt}_{hh}_{pj}")
                    nc.tensor.matmul(ps2[:, 0, lo0:512],
                                     kT_t[kb0 // 4][:, (kb0 % 4) * P:
                                                    (kb0 % 4 + 1) * P],
                                     qr_tiles[tt][hh][:, lo0:512],
                                     start=True, stop=True)
                    nc.tensor.matmul(ps2[:, 1, lo1:512],
                                     kT_t[kb1 // 4][:, (kb1 % 4) * P:
                                                    (kb1 % 4 + 1) * P],
                                     qr_tiles[tt][hh][:, lo1:512],
                                     start=True, stop=True)
                    e2 = e_pool.tile([P, 2, 512], CDT, tag="e",
                                     name=f"e{tt}_{hh}_{pj}")
                    # single exp over [lo0:512] for both parities; on diag
                    # pairs parity-1's [lo0:lo1) is garbage no consumer reads
                    nc.scalar.activation(e2[:, :, lo0:512], ps2[:, :, lo0:512],
                                         AF.Exp, scale=SCALE)
                    if d0 >= 0:
                        with nc.allow_low_precision(reason="mask mult"):
                            nc.vector.tensor_mul(e2[:, 0, d0 * P:(d0 + 1) * P],
                                                 e2[:, 0, d0 * P:(d0 + 1) * P],
                                                 tri_sb[:])
                    if d1 >= 0:
                        with nc.allow_low_precision(reason="mask mult"):
                            nc.vector.tensor_mul(e2[:, 1, d1 * P:(d1 + 1) * P],
                                                 e2[:, 1, d1 * P:(d1 + 1) * P],
                                                 tri_sb[:])
                    # denominator accumulation on DVE (fp16)
                    with nc.allow_low_precision(reason="den acc"):
                        if pj == 0:
                            nc.vector.tensor_add(acc[:, lo1:512],
                                                 e2[:, 0, lo1:512],
                                                 e2[:, 1, lo1:512])
                            if lo1 > lo0:
                                nc.vector.tensor_copy(acc[:, lo0:lo1],
                                                      e2[:, 0, lo0:lo1])
                        else:
                            nc.vector.tensor_add(acc[:, lo0:512],
                                                 acc[:, lo0:512],
                                                 e2[:, 0, lo0:512])
                            nc.vector.tensor_add(acc[:, lo1:512],
                                                 acc[:, lo1:512],
                                                 e2[:, 1, lo1:512])
                    pop_fill(per_pop)
                    if len(pend) >= 2:
                        pk0, pl0, pk1, pl1, pe, first = pend.pop(0)
                        nc.tensor.matmul(o_ps[:, pl0:512],
                                         v_t[pk0 // 4][:, pk0 % 4, :],
                                         pe[:, 0, pl0:512],
                                         start=first, stop=False)
                        nc.tensor.matmul(o_ps[:, pl1:512],
                                         v_t[pk1 // 4][:, pk1 % 4, :],
                                         pe[:, 1, pl1:512],
                                         start=False, stop=False)
                    pend.append((kb0, lo0, kb1, lo1, e2, pj == 0))
                    pop_fill(per_pop)
                while pend:
                    pk0, pl0, pk1, pl1, pe, first = pend.pop(0)
                    last = not pend
                    nc.tensor.matmul(o_ps[:, pl0:512],
                                     v_t[pk0 // 4][:, pk0 % 4, :],
                                     pe[:, 0, pl0:512],
                                     start=first, stop=False)
                    nc.tensor.matmul(o_ps[:, pl1:512],
                                     v_t[pk1 // 4][:, pk1 % 4, :],
                                     pe[:, 1, pl1:512],
                                     start=False, stop=last)
                    if not last:
                        pop_fill(per_pop)
                while due:
                    due.pop(0)()

                def mk_due(o_ps=o_ps, acc=acc, hh=hh, ot_sb=ot_sb, tt=tt):
                    cell = {}

                    def den_fn():
                        # denominator broadcast matmul (rides the pp ring)
                        den_ps = pp.tile([P, 2, 512], F32, tag="pair",
                                         name=f"d{tt}_{hh}")
                        nc.tensor.matmul(den_ps[:, 0, :], ones_mat, acc[:],
                                         start=True, stop=True)
                        bc_sb = bc_pool.tile([P, 512], F32, tag="bc",
                                             name=f"bs{tt}_{hh}")
                        if tt < 3:
                            nc.scalar.copy(bc_sb[:], den_ps[:, 0, :])
                        else:
                            nc.vector.tensor_copy(bc_sb[:], den_ps[:, 0, :])
                        cell['bc'] = bc_sb

                    def fin_fn():
                        finish2(cell['bc'], o_ps, hh, ot_sb)
                    return [den_fn, fin_fn]
                due = mk_due()
                pop_fill(per_pop)
            return due

        # ---------------- main schedule ----------------
        due = []
        a3_q23 = []
        for tt in range(NT):
            if tt + 2 < NT:
                load_x_tile(tt + 2)
            if tt == 0:
                fillers.extend(a0_tail_items())
            if tt + 1 < NT:
                if tt + 1 == 3:
                    main3, a3_q23 = a_items(3, split_q23=True)
                    fillers.extend(main3)
                else:
                    fillers.extend(a_items(tt + 1))
            else:
                fillers.extend(a3_q23)
                fillers.extend(c_items(2, pa))
            due = phase_b(tt, due)
            # drain leftovers interleaved with the last head's den/norm
            while fillers and due:
                fillers.pop(0)[1]()
                due.pop(0)()
            while fillers:
                fillers.pop(0)[1]()
            while due:
                due.pop(0)()
            due = []
            if tt in (0, 1):
                for _, fn in c_items(tt, pp):
                    fn()
            if tt == 3:
                for _, fn in c_items(3, pp, eager_tail=True):
                    fn()
                for i in range(24):
                    dmy = po.tile([P, 512], F32, tag="o", name=f"dmyz{i}")
                    nc.tensor.matmul(dmy[:, 0:128], dum_sb[:, 0:128],
                                     dum_sb[:, 0:128], start=True, stop=True,
                                     skip_group_check=True)
    nc.compile()
    return nc


def _host_tables():
    freqs = (1.0 / (np.float32(10000.0) **
                    (np.arange(0, HD, 2, dtype=np.float32) / np.float32(HD)))).astype(np.float32)
    t = np.arange(T, dtype=np.float32)
    ang = t[:, None] * freqs[None, :]
    cos = np.tile(np.cos(ang), (1, 2)).astype(np.float32)   # (T, HD)
    sin = np.tile(np.sin(ang), (1, 2)).astype(np.float32)
    cosT = np.ascontiguousarray(cos.T)                       # (HD, T)
    sinT = np.ascontiguousarray(sin.T)
    ssinT = sinT.copy()
    ssinT[:HD // 2] *= -1.0                                  # sign-folded sin
    rmat = np.zeros((P, P), dtype=np.float32)
    for j in range(HD // 2):
        rmat[j + HD // 2, j] = 1.0
    for j in range(HD // 2, HD):
        rmat[j - HD // 2, j] = 1.0
    tri = (np.arange(P)[:, None] <= np.arange(P)[None, :]).astype(np.float32)
    ident = np.eye(P, dtype=np.float32)
    return cosT, ssinT, rmat, tri, ident


def _make_in_maps(x, wq, wk, wv, wo):
    cosT, ssinT, rmat, tri, ident = _host_tables()
    x = np.asarray(x, dtype=np.float32)
    wq = np.asarray(wq, dtype=np.float32)
    wk = np.asarray(wk, dtype=np.float32)
    wv = np.asarray(wv, dtype=np.float32)
    wo = np.asarray(wo, dtype=np.float32)

    in_maps = []
    for c in range(8):
        b, h = divmod(c, 4)
        xTb = x[b].T.reshape(4, 4, P, NT, 512)          # (g, dc, p, tt, t)
        xS = np.ascontiguousarray(xTb.transpose(3, 0, 2, 1, 4))  # (tt,g,p,dc,t)
        wqS = np.ascontiguousarray(
            wq[:, h * 512:(h + 1) * 512].reshape(4, 4, P, 512).transpose(0, 2, 1, 3))
        wkS = np.ascontiguousarray(
            wk[:, h * HD:(h + 1) * HD].reshape(DC, P, HD).transpose(1, 0, 2))
        wvS = np.ascontiguousarray(
            wv[:, h * HD:(h + 1) * HD].reshape(DC, P, HD).transpose(1, 0, 2))
        in_maps.append({
            "xT": xS.astype(NPDT),
            "wq": wqS.astype(NPDT),
            "wk": wkS.astype(NPDT),
            "wv": wvS.astype(NPDT),
            "wo": np.ascontiguousarray(wo[h * 512:(h + 1) * 512, :]).astype(NPDT),
            "cosT": cosT.astype(NPDT), "ssinT": ssinT.astype(NPDT),
            "rmat": rmat.astype(NPDT), "tri": tri.astype(NPDT),
            "ident": ident.astype(NPDT),
        })
    return in_maps


def kernel(x, wq, wk, wv, wo):
    if "nc" not in _cached:
        _cached["nc"] = _build()
    nc = _cached["nc"]
    in_maps = _make_in_maps(x, wq, wk, wv, wo)
    try:
        res = run_bass_kernel_spmd(nc, in_maps, core_ids=list(range(8)))
    except Exception:
        # transient NRT/device hiccups recover on a clean retry
        res = run_bass_kernel_spmd(nc, in_maps, core_ids=list(range(8)))
    outs = [res.results[c]["out"].astype(np.float32) for c in range(8)]
    full = np.stack([outs[0] + outs[1] + outs[2] + outs[3],
                     outs[4] + outs[5] + outs[6] + outs[7]], axis=0)
    return full.astype(np.float32)
